# revision 16
# baseline (speedup 1.0000x reference)
"""DiceLoss kernel for Trainium2 (8 NeuronCores, pure data parallel).

Problem: softmax over C=19 classes of predict [8, 19, 512, 512], one-hot of
target [8, 512, 512], then per-sample per-class sums
    psum[n,c]  = sum_pix softmax(x)[n,c,pix]
    inter[n,c] = sum_{pix: t=c} softmax(x)[n,c,pix]
    tsum[n,c]  = #{pix: t=c}
and dice = mean_c mean_n (1 - (2*inter+1)/(psum+tsum+1)).

Sharding: one sample per core (batch N=8 across 8 cores).

Key trick: the HOST SORTS each sample's pixels by target class (padding each
class's run to a whole 128-pixel column; pad pixels get a one-hot x row so
their softmax contribution is an exactly-known integer the host subtracts).
This kills the device-side one-hot masks, the mask multiply and the whole
`t` tensor: the device only produces per-class per-column sums of softmax
(full column resolution, [19, FTOTP] fp32), and the host reduces them into
psum (all columns) and inter (each class's own column range) - pure index
arithmetic on device-computed sums.

Device layout per core: x as [C, 128, FTOTP] fp8-e4m3 (pixel-partition,
class-blocked free dim; sorted pixel k sits at partition k%128, column
k//128), processed in column chunks (256,512,512,512,256,tail - small edge
chunks shrink pipeline fill/drain):
  - ScalarE: Exp activation only (two class-group halves per chunk,
    pipelined behind the split DMA).
  - DVE: pairwise-tree class sum -> denominator (bf16 2x mode; the sb/sd
    tree branches and 3 of 19 normalize planes are offloaded to the
    otherwise-idle Pool engine), reciprocal via bitcast-magic seed + one
    Newton step (the RECIPROCAL instruction measures ~8 cycles/elem, the
    Newton sequence ~1.5), and the wide in-place normalize et *= R.
  - Pool (gpsimd): sb/sd tree adds + last 3 classes of the normalize
    (~2ns/elem measured - the cost-model's Add/Multiply efficiency holds).
  - TensorE: per class a [128,1] all-ones lhsT matmul accumulates the
    pixel-partition column sums of prob into row c of the chunk's [19, Fj]
    PSUM bank; each bank is DMA'd straight to DRAM when its chunk stops.
tsum is the exact integer histogram of the target input, computed on host
during sharding; psum gets the known pad contribution subtracted on host.

Hardware quirks worked around here: at most ONE sync-wait per instruction
-> custom tail drain + body legalized by bass_rust.generate_event_semaphores;
all DMAs via SP HWDGE; gpsimd does constant memsets + its tensor-op share.
"""

import numpy as np
import ml_dtypes

N, C, H, W = 8, 19, 512, 512
PIX = H * W  # 262144
P = 128
NCORES = 8
RMAGIC = 0x7EF1  # bf16 reciprocal seed: bits(1/x) ~= RMAGIC - bits(x)
MSPLIT = 16  # normalize: classes [0,16) on DVE, [16,19) on Pool

_PROGS = {}


def _chunks_of(ftotp):
    tail = ftotp - 2048
    assert 2 <= tail
    return [128, 384, 512, 512, 512, tail]


def _build_program(ftotp):
    from contextlib import ExitStack

    import concourse.bass as bass
    import concourse.tile as tile
    from concourse import mybir

    dt = mybir.dt
    Alu = mybir.AluOpType
    Act = mybir.ActivationFunctionType

    import bass_rust as _br

    class _TC(tile.TileContext):
        # Stock Tile puts one sem-wait per active proc on the tail drain,
        # which this walrus rejects (>1 wait per instruction). Emit the
        # global-clock waits as single-wait drains instead; body
        # instructions are legalized by bass_rust.generate_event_semaphores
        # after the context exits.
        def _drain_and_barrier(self, tick_clock, wait_clock):
            from concourse.vector_clock import ScopedClock

            nc = self.nc
            drain_inst = nc.sync.drain()
            wait_clock.add_sem_waits(
                drain_inst.ins, ScopedClock({None: tick_clock.global_clock})
            )
            si = drain_inst.ins.sync_info
            moved = []
            while len(si.on_wait) > 1:
                moved.append(si.on_wait.pop())
            for w in moved:
                d2 = nc.sync.drain()
                d2.ins.sync_info = _br.SyncInfo(on_wait=[w], on_update=[])

            nc.all_engine_barrier()
            assert self.sems is not None
            popped = nc._tile_sem_poison_stack.pop()
            assert popped is self._sem_poison
            nc.clear_and_free_semaphores(list(self.sems.allocated().values()))
            nc.all_engine_barrier()

    chunks = _chunks_of(ftotp)
    NB = len(chunks)

    nc = bass.Bass(
        "TRN2", target_bir_lowering=False, debug=False, num_devices=NCORES
    )
    x_d = nc.dram_tensor("x", [C, P, ftotp], dt.float8e4, kind="ExternalInput").ap()
    out_d = nc.dram_tensor("out", [C, ftotp], dt.float32, kind="ExternalOutput").ap()

    with nc.allow_low_precision("bf16 softmax-stat kernel"), \
            _TC(nc) as tc, ExitStack() as ctx:
        xp = ctx.enter_context(tc.tile_pool(name="xp", bufs=4))
        ep = ctx.enter_context(tc.tile_pool(name="ep", bufs=3))
        sp = ctx.enter_context(tc.tile_pool(name="sp", bufs=3))
        dp = ctx.enter_context(tc.tile_pool(name="dp", bufs=2))
        cp = ctx.enter_context(tc.tile_pool(name="cp", bufs=1))
        pp = ctx.enter_context(tc.tile_pool(name="pp", bufs=1, space="PSUM"))

        # per-class one-hot lhsT columns: block c is a [P, C] matrix whose
        # column c is all-ones -> matmul with rhs [P, F] lands the
        # pixel-partition column sums of rhs on PSUM partition c.
        cols = cp.tile([P, C * C], dt.bfloat16)
        nc.gpsimd.memset(cols[:], 0.0)
        for c in range(C):
            nc.gpsimd.memset(cols[:, c * C + c : c * C + c + 1], 1.0)
        # reciprocal magic constant tile (uint16)
        ku = cp.tile([P, 512], dt.uint16)
        nc.gpsimd.memset(ku[:], float(RMAGIC))

        banks = [pp.tile([C, fj], dt.float32, name=f"bank{j}")
                 for j, fj in enumerate(chunks)]
        stages = [cp.tile([C, fj], dt.float32, name=f"stage{j}")
                  for j, fj in enumerate(chunks)]

        CSPLIT = 10
        colbase = 0
        for j, fj in enumerate(chunks):
            xt = xp.tile([P, C * fj], dt.float8e4, tag="x")
            xv = xt[:].rearrange("p (c f) -> p c f", c=C)
            et = ep.tile([P, C * fj], dt.bfloat16, tag="e")
            ev = et[:].rearrange("p (c f) -> p c f", c=C)
            for c0, c1 in ((0, CSPLIT), (CSPLIT, C)):
                nc.sync.dma_start(
                    out=xv[:, c0:c1, :],
                    in_=x_d[c0:c1, :, colbase : colbase + fj].rearrange(
                        "c p f -> p c f"
                    ),
                )
                nc.scalar.activation(
                    et[:, c0 * fj : c1 * fj], xt[:, c0 * fj : c1 * fj], Act.Exp
                )

            # denominator tree: level 1 split by exp half so the first-10
            # pairs run while exp of classes 10-18 is still going. sb/sd go
            # to the Pool engine, the rest to DVE.
            sa = sp.tile([P, 5 * fj], dt.bfloat16, tag="sa", bufs=1)
            sav = sa[:].rearrange("p (c f) -> p c f", c=5)
            nc.vector.tensor_tensor(
                sav[:, :, :], ev[:, 0:10:2, :], ev[:, 1:10:2, :], Alu.add
            )
            sb = sp.tile([P, 4 * fj], dt.bfloat16, tag="sb", bufs=1)
            sbv = sb[:].rearrange("p (c f) -> p c f", c=4)
            nc.gpsimd.tensor_tensor(
                sbv[:, :, :], ev[:, 10:18:2, :], ev[:, 11:19:2, :], Alu.add
            )
            sc = sp.tile([P, 2 * fj], dt.bfloat16, tag="sc", bufs=1)
            scv = sc[:].rearrange("p (c f) -> p c f", c=2)
            nc.vector.tensor_tensor(
                scv[:, :, :], sav[:, 0:4:2, :], sav[:, 1:5:2, :], Alu.add
            )
            sd = sp.tile([P, 2 * fj], dt.bfloat16, tag="sd", bufs=1)
            sdv = sd[:].rearrange("p (c f) -> p c f", c=2)
            nc.gpsimd.tensor_tensor(
                sdv[:, :, :], sbv[:, 0:4:2, :], sbv[:, 1:4:2, :], Alu.add
            )
            se = sp.tile([P, fj], dt.bfloat16, tag="se", bufs=1)
            nc.vector.tensor_tensor(se[:], scv[:, 0, :], scv[:, 1, :], Alu.add)
            sf = sp.tile([P, fj], dt.bfloat16, tag="sf", bufs=1)
            nc.vector.tensor_tensor(sf[:], sdv[:, 0, :], sdv[:, 1, :], Alu.add)
            d0 = sp.tile([P, fj], dt.bfloat16, tag="d0", bufs=1)
            nc.vector.tensor_tensor(d0[:], se[:], sf[:], Alu.add)
            d1 = sp.tile([P, fj], dt.bfloat16, tag="d1", bufs=1)
            nc.vector.tensor_tensor(d1[:], d0[:], sav[:, 4, :], Alu.add)
            dd = sp.tile([P, fj], dt.bfloat16, tag="dd", bufs=1)
            nc.vector.tensor_tensor(dd[:], d1[:], ev[:, 18, :], Alu.add)

            # reciprocal: bitcast magic seed + one bf16 Newton step
            r0 = dp.tile([P, fj], dt.bfloat16, tag="r0")
            nc.vector.tensor_tensor(
                r0[:].bitcast(dt.uint16), ku[:, 0:fj], dd[:].bitcast(dt.uint16),
                Alu.subtract,
            )
            yt = dp.tile([P, fj], dt.bfloat16, tag="yt")
            nc.vector.tensor_tensor(yt[:], dd[:], r0[:], Alu.mult)
            zt = dp.tile([P, fj], dt.bfloat16, tag="zt")
            nc.vector.tensor_scalar(zt[:], yt[:], -1.0, 2.0, Alu.mult, Alu.add)
            rt = dp.tile([P, fj], dt.bfloat16, tag="rt")
            nc.vector.tensor_tensor(rt[:], zt[:], r0[:], Alu.mult)

            # wide in-place normalize et *= R (broadcast over classes),
            # in two DVE halves so the first matmuls start early
            rb10 = rt[:].rearrange("p (o f) -> p o f", o=1).broadcast_to(
                (P, 10, fj)
            )
            nc.vector.tensor_tensor(
                ev[:, 0:10, :], ev[:, 0:10, :], rb10, Alu.mult
            )
            rb9 = rt[:].rearrange("p (o f) -> p o f", o=1).broadcast_to(
                (P, 9, fj)
            )
            nc.vector.tensor_tensor(
                ev[:, 10:19, :], ev[:, 10:19, :], rb9, Alu.mult
            )

            for c in range(C):
                nc.tensor.matmul(
                    banks[j][:],
                    lhsT=cols[:, c * C : (c + 1) * C],
                    rhs=et[:, c * fj : (c + 1) * fj],
                    start=(c == 0),
                    stop=(c == C - 1),
                )
            nc.scalar.activation(stages[j][:], banks[j][:], Act.Copy)
            nc.scalar.dma_start(
                out=out_d[:, colbase : colbase + fj], in_=stages[j][:]
            )
            colbase += fj

    _br.move_matmul_waits_to_ldweights(nc.m)
    _br.generate_event_semaphores(nc)
    return nc


def _get_program(ftotp):
    if ftotp not in _PROGS:
        _PROGS[ftotp] = _build_program(ftotp)
    return _PROGS[ftotp]


PAD_NEG = -100.0


def _shard_inputs(predict, target):
    """Sort each sample's pixels by target class, pad each class run to a
    whole 128-pixel column, build the device layout.

    Returns (in_maps, counts [N,C], padcnt [N,C], masks [N,C,ftotp], ftotp).
    """
    x = np.ascontiguousarray(predict, dtype=np.float32).reshape(N, C, PIX)
    t = np.ascontiguousarray(target).reshape(N, PIX).astype(np.int64)

    counts = np.stack([np.bincount(t[i], minlength=C)[:C] for i in range(N)])
    ncols = -(-counts // P)  # ceil per class
    total_cols = ncols.sum(axis=1)
    ftotp = int(max(int(total_cols.max()), 2050))
    if ftotp % 2:
        ftotp += 1

    in_maps = []
    padcnt = np.zeros((N, C), dtype=np.float32)
    masks = np.zeros((N, C, ftotp), dtype=np.float32)
    for i in range(N):
        order = np.argsort(t[i], kind="stable")
        xs = x[i][:, order]  # [C, PIX] class-sorted pixel columns
        dst = np.full((C, ftotp * P), PAD_NEG, dtype=np.float32)
        pos = 0
        src = 0
        for c in range(C):
            n = int(counts[i, c])
            dst[:, pos : pos + n] = xs[:, src : src + n]
            nc_c = int(ncols[i, c])
            pad = nc_c * P - n
            if pad:
                pc = (c + 1) % C
                dst[pc, pos + n : pos + nc_c * P] = 0.0
                padcnt[i, pc] += pad
            masks[i, c, pos // P : pos // P + nc_c] = 1.0
            pos += nc_c * P
            src += n
        tailpix = ftotp * P - pos
        if tailpix:
            dst[0, pos:] = 0.0
            padcnt[i, 0] += tailpix
        xdev = np.ascontiguousarray(
            dst.reshape(C, ftotp, P).transpose(0, 2, 1)
        ).astype(ml_dtypes.float8_e4m3fn)
        in_maps.append({"x": xdev})
    return in_maps, counts.astype(np.float32), padcnt, masks, ftotp


def kernel(predict, target):
    from concourse.bass_utils import run_bass_kernel_spmd

    in_maps, counts, padcnt, masks, ftotp = _shard_inputs(predict, target)
    nc = _get_program(ftotp)
    res = run_bass_kernel_spmd(nc, in_maps, list(range(NCORES)))
    colsums = np.stack(
        [
            np.asarray(res.results[i]["out"], dtype=np.float32).reshape(C, ftotp)
            for i in range(NCORES)
        ]
    )
    psum = colsums.sum(axis=2) - padcnt
    inter = (colsums * masks).sum(axis=2)
    tsum = counts
    top = 2.0 * inter + 1.0
    bot = psum + tsum + 1.0
    per_class = np.mean(1.0 - top / bot, axis=0, dtype=np.float32)
    return np.float32(per_class.sum() / C)
